# revision 2
# baseline (speedup 1.0000x reference)
"""Cubic B-spline kernel v5: shared-run gather, DVE-contiguous table layout (Trainium2 Bass/Tile, 8 cores).

The working indirect-DMA form costs ~2.37us per instruction (128
descriptors, one offset per partition) regardless of run size, so the
per-point-descriptor baseline is instruction-bound at 1960 instr =
4.66ms.  v4 cuts descriptors ~4x by sharing one contiguous table run
among all points in a (cell, z-third) bucket:

  - Table T16b (bf16): T16b[x0][y0][z][i][j] = G[x0+i][y0+j][z],
    z padded to 132 slots; cell row = 132*16 = 2112 bf16.
  - Bucket = (cell = (x0,y0), third t3 = min(z0//43, 2)).  Run = slots
    43*t3 .. 43*t3+45 (46 slots = 736 bf16 = 1472B), which covers every
    window z0..z0+3 with z0 in the third.  lambda = 5.13 points/bucket.
  - Buckets are packed host-side into groups of up to 6 point-slots;
    each group = one 736-el descriptor (the n1 indirect form).  63,488
    group capacity = 62 chunks x 128 partitions x 8 groups -> 496
    gather instructions (~1.2ms Pool DGE).
  - Per-point z-window extraction becomes a dense 46-slot weighted
    reduction: wz[s] = N(zf - s) with N the cardinal cubic B-spline
    kernel, evaluated arithmetically on DVE (N(t) = (a^3 - 4b^3)/6,
    a = max(2-|t|,0), b = max(1-|t|,0)); zeros outside the window make
    extraction implicit.  Then the usual (i,j) tensor-product contract.
  - Host packs buckets/groups, ships pts+zbase (f32 [*,4]) + int32 run
    offsets, unshards by inverse permutation; numpy fallback for
    capacity overflow (6.5-sigma padded, never expected to fire).
"""

from contextlib import ExitStack

import numpy as np

GRID = 132
G2 = GRID * GRID
P = 128

NZP = 132
CELL = 16
ROW = 3 * 16 * 46          # 2208 bf16 per cell: [t3][ij][s] blocks
PLANE = 128 * ROW
TSIZE = 128 * PLANE

TH = 43                    # z0 thirds: [0,43), [43,86), [86,128)
RUN = 46 * CELL            # 736 els = 1472B per descriptor
G = 6                      # point-slots per group
NCHUNK = 62
NGRP = NCHUNK * P * 8      # 63488 groups
TOT = NCHUNK * 48          # out columns per partition (8 groups x 6 slots)
NSLOT = NGRP * G           # 380928
NPTS_CORE = 250_000

_CACHE = {}


def _build_program(nreps=1):
    import concourse.bass as bass
    import concourse.tile as tile
    from concourse import bacc, mybir

    nc = bacc.Bacc("TRN2", num_devices=8, debug=False, target_bir_lowering=False)
    pts_d = nc.dram_tensor("pts4", [P * TOT, 4], mybir.dt.float32, kind="ExternalInput")
    idx_d = nc.dram_tensor("idx", [NGRP, 1], mybir.dt.int32, kind="ExternalInput")
    gt_d = nc.dram_tensor("gt", [GRID * G2, 1], mybir.dt.float32, kind="ExternalInput")
    iota_d = nc.dram_tensor("iota", [P, 46], mybir.dt.float32, kind="ExternalInput")
    out_d = nc.dram_tensor("out", [P, TOT], mybir.dt.float32, kind="ExternalOutput")
    t16_d = nc.dram_tensor("t16", [TSIZE, 1], mybir.dt.bfloat16, kind="Internal")

    f32 = mybir.dt.float32
    bf16 = mybir.dt.bfloat16
    AL = mybir.AluOpType

    def sap(ap, pattern, off=0):
        v = ap.copy()
        v.ap = type(v.ap)(pattern)
        v.offset = v.offset + off
        return v

    with tile.TileContext(nc) as tc:
        with ExitStack() as ctx:
            bpool = ctx.enter_context(tc.tile_pool(name="bpool", bufs=1))
            cpool = ctx.enter_context(tc.tile_pool(name="cpool", bufs=1))
            pool = ctx.enter_context(tc.tile_pool(name="pool", bufs=2))
            tpool = ctx.enter_context(tc.tile_pool(name="tpool", bufs=1))
            xpool = ctx.enter_context(tc.tile_pool(name="xpool", bufs=2))

            # ---- iota constant (loaded once) ----
            iota_t = cpool.tile([P, 46], f32, tag="iota")
            nc.sync.dma_start(iota_t[:], iota_d[:])

            # ---- phase 1: build T16b (bf16, z-padded) ----
            for blk in range(32):
                x0b = blk * 4
                gtiles = []
                for j in range(4):
                    gj = bpool.tile([P, 7, GRID], f32, tag=f"g{j}")
                    src = sap(
                        gt_d[:],
                        [[G2, P], [GRID, 7], [1, GRID]],
                        j * G2 + x0b * GRID,
                    )
                    nc.sync.dma_start(gj[:], src)
                    gtiles.append(gj)
                S = bpool.tile([P, 4, 3, 16, 46], bf16, tag="S")
                for t3 in range(3):
                    for i in range(4):
                        for j in range(4):
                            nc.vector.tensor_copy(
                                S[:, :, t3, i * 4 + j, :],
                                gtiles[j][:, i : i + 4,
                                          TH * t3 : TH * t3 + 46],
                            )
                for b in range(4):
                    dst = sap(t16_d[:], [[ROW, P], [1, ROW]], (x0b + b) * PLANE)
                    nc.sync.dma_start(dst, S[:, b])

            # ---- phase 2 ----
            for rep in range(nreps):
                for ch in range(NCHUNK):
                    idxi = pool.tile([P, 8], mybir.dt.int32, tag="idx")
                    nc.sync.dma_start(
                        idxi[:], sap(idx_d[:], [[8, P], [1, 8]], ch * 1024)
                    )
                    R = xpool.tile([P, 8, RUN], bf16, tag="R")
                    for g in range(8):
                        nc.gpsimd.indirect_dma_start(
                            out=R[:, g],
                            out_offset=None,
                            in_=t16_d[:],
                            in_offset=bass.IndirectOffsetOnAxis(
                                ap=idxi[:, g : g + 1], axis=0
                            ),
                        )

                    pts4 = pool.tile([P, 8, G, 4], f32, tag="pts4")
                    nc.sync.dma_start(
                        pts4[:],
                        sap(pts_d[:], [[TOT * 4, P], [24, 8], [4, G], [1, 4]],
                            ch * 192),
                    )

                    # --- x/y spline weights on [P, 48, 2] views of pts4 ---
                    TOT_AP = 8 * G * 4
                    pxy = sap(pts4[:], [[TOT_AP, P], [4, 48], [1, 2]])
                    t_t = pool.tile([P, 48, 2], f32, tag="t")
                    nc.vector.tensor_scalar_add(t_t[:], pxy, 1.0)
                    r_t = pool.tile([P, 48, 2], f32, tag="r")
                    nc.vector.tensor_scalar(
                        r_t[:], t_t[:], 8388608.0, 8388608.0,
                        op0=AL.add, op1=AL.subtract,
                    )
                    gt_t = pool.tile([P, 48, 2], f32, tag="gtt")
                    nc.vector.tensor_tensor(gt_t[:], r_t[:], t_t[:], op=AL.is_gt)
                    tif_t = pool.tile([P, 48, 2], f32, tag="tif")
                    nc.vector.tensor_sub(tif_t[:], r_t[:], gt_t[:])
                    frac = pool.tile([P, 48, 2], f32, tag="frac")
                    nc.vector.tensor_sub(frac[:], t_t[:], tif_t[:])

                    W = pool.tile([P, 48, 2, 4], f32, tag="W")
                    omx = pool.tile([P, 48, 2], f32, tag="omx")
                    nc.vector.tensor_scalar(
                        omx[:], frac[:], -1.0, -1.0, op0=AL.mult, op1=AL.subtract
                    )
                    x2 = pool.tile([P, 48, 2], f32, tag="x2")
                    nc.vector.tensor_mul(x2[:], frac[:], frac[:])
                    x3 = pool.tile([P, 48, 2], f32, tag="x3")
                    nc.vector.tensor_mul(x3[:], x2[:], frac[:])
                    o2 = pool.tile([P, 48, 2], f32, tag="o2")
                    nc.vector.tensor_mul(o2[:], omx[:], omx[:])
                    o3 = pool.tile([P, 48, 2], f32, tag="o3")
                    nc.vector.tensor_mul(o3[:], o2[:], omx[:])
                    SIX = 1.0 / 6.0
                    nc.vector.tensor_scalar_mul(W[:, :, :, 0], o3[:], SIX)
                    nc.vector.tensor_scalar_mul(W[:, :, :, 3], x3[:], SIX)
                    c1a = pool.tile([P, 48, 2], f32, tag="c1a")
                    nc.vector.scalar_tensor_tensor(
                        c1a[:], x3[:], 0.5, x2[:], op0=AL.mult, op1=AL.subtract
                    )
                    nc.vector.tensor_scalar_add(W[:, :, :, 1], c1a[:], 2.0 / 3.0)
                    c2a = pool.tile([P, 48, 2], f32, tag="c2a")
                    nc.vector.scalar_tensor_tensor(
                        c2a[:], o3[:], 0.5, o2[:], op0=AL.mult, op1=AL.subtract
                    )
                    nc.vector.tensor_scalar_add(W[:, :, :, 2], c2a[:], 2.0 / 3.0)

                    # wxy[p, 48, i, j] = wx[i] * wy[j]
                    wxy = pool.tile([P, 48, 4, 4], f32, tag="wxy")
                    Wap = 48 * 8
                    wxv = sap(W[:], [[Wap, P], [8, 48], [1, 4], [0, 4]], 0)
                    wyv = sap(W[:], [[Wap, P], [8, 48], [0, 4], [1, 4]], 4)
                    nc.vector.tensor_tensor(wxy[:], wxv, wyv, op=AL.mult)

                    # --- z weights: wz[p, 48, 46] = N(zf - s) ---
                    zf = pool.tile([P, 48], f32, tag="zf")
                    pzv = sap(pts4[:], [[TOT_AP, P], [4, 48]], 2)
                    zbv = sap(pts4[:], [[TOT_AP, P], [4, 48]], 3)
                    nc.vector.tensor_sub(zf[:], pzv, zbv)

                    d = tpool.tile([P, 48, 46], f32, tag="d")
                    zfb = sap(zf[:], [[48, P], [1, 48], [0, 46]])
                    iob = sap(iota_t[:], [[46, P], [0, 48], [1, 46]])
                    nc.vector.tensor_tensor(d[:], zfb, iob, op=AL.subtract)
                    nd = tpool.tile([P, 48, 46], f32, tag="nd")
                    nc.vector.tensor_scalar_mul(nd[:], d[:], -1.0)
                    u = tpool.tile([P, 48, 46], f32, tag="u")
                    nc.vector.tensor_max(u[:], d[:], nd[:])
                    a = tpool.tile([P, 48, 46], f32, tag="a")
                    nc.vector.tensor_scalar(
                        a[:], u[:], -1.0, 2.0, op0=AL.mult, op1=AL.add
                    )
                    nc.vector.tensor_scalar_max(a[:], a[:], 0.0)
                    b = tpool.tile([P, 48, 46], f32, tag="b")
                    nc.vector.tensor_scalar(
                        b[:], u[:], -1.0, 1.0, op0=AL.mult, op1=AL.add
                    )
                    nc.vector.tensor_scalar_max(b[:], b[:], 0.0)
                    a2 = tpool.tile([P, 48, 46], f32, tag="a2")
                    nc.vector.tensor_mul(a2[:], a[:], a[:])
                    a3 = tpool.tile([P, 48, 46], f32, tag="a3")
                    nc.vector.tensor_mul(a3[:], a2[:], a[:])
                    b2 = tpool.tile([P, 48, 46], f32, tag="b2")
                    nc.vector.tensor_mul(b2[:], b[:], b[:])
                    b3s = tpool.tile([P, 48, 46], f32, tag="b3s")
                    nc.vector.scalar_tensor_tensor(
                        b3s[:], b2[:], 2.0 / 3.0, b[:], op0=AL.mult, op1=AL.mult
                    )
                    wzf = tpool.tile([P, 48, 46], f32, tag="wzf")
                    nc.vector.scalar_tensor_tensor(
                        wzf[:], a3[:], SIX, b3s[:], op0=AL.mult, op1=AL.subtract
                    )
                    wzb = tpool.tile([P, 48, 46], bf16, tag="wzb")
                    nc.vector.tensor_copy(wzb[:], wzf[:])

                    # --- per-group dense z-contraction ---
                    Y = pool.tile([P, 8, G, 16], f32, tag="Y")
                    for g in range(8):
                        m = tpool.tile([P, G, 16, 46], bf16, tag="m")
                        Rv = sap(R[:], [[8 * RUN, P], [0, G], [46, 16], [1, 46]],
                                 g * RUN)
                        wv = sap(wzb[:], [[48 * 46, P], [46, G], [0, 16], [1, 46]],
                                 g * G * 46)
                        nc.vector.tensor_tensor(m[:], Rv, wv, op=AL.mult)
                        nc.vector.tensor_reduce(
                            Y[:, g].rearrange("p s e -> p (s e)"),
                            m[:].rearrange("p s e z -> p (s e) z"),
                            axis=mybir.AxisListType.X,
                            op=AL.add,
                        )

                    m2 = pool.tile([P, 48, 16], f32, tag="m2")
                    Yv = sap(Y[:], [[48 * 16, P], [16, 48], [1, 16]])
                    wxyv = sap(wxy[:], [[48 * 16, P], [16, 48], [1, 16]])
                    nc.vector.tensor_tensor(m2[:], Yv, wxyv, op=AL.mult)
                    v = pool.tile([P, 48], f32, tag="v")
                    nc.vector.tensor_reduce(
                        v[:], m2[:], axis=mybir.AxisListType.X, op=AL.add
                    )

                    dst = sap(out_d[:], [[TOT, P], [1, 48]], ch * 48)
                    nc.sync.dma_start(dst, v[:])

    nc.compile()
    return nc


def _host_pack(pts):
    """Pack one core's points into groups. Returns device arrays + meta."""
    t = pts + np.float32(1.0)
    ti = np.trunc(t).astype(np.int64)
    x0, y0, z0 = ti[:, 0] - 1, ti[:, 1] - 1, ti[:, 2] - 1
    cell = x0 * 128 + y0
    t3 = np.minimum(z0 // TH, 2)
    bucket = cell * 3 + t3
    NBUK = 16384 * 3

    order = np.argsort(bucket, kind="stable")
    bsort = bucket[order]
    counts = np.bincount(bucket, minlength=NBUK)
    gb = -(-counts // G)                      # groups per bucket
    base = np.zeros(NBUK + 1, np.int64)
    np.cumsum(gb, out=base[1:])
    total_groups = int(base[-1])

    overflow = np.array([], np.int64)
    if total_groups > NGRP:
        # drop whole buckets from the end; their points go to host fallback
        cut = int(np.searchsorted(base[1:], NGRP, side="right"))
        drop = np.isin(bucket, np.arange(cut, NBUK)[gb[cut:] > 0])
        keep_bucket = bucket < cut
        overflow = np.nonzero(~keep_bucket)[0]
        order = order[keep_bucket[order]]
        bsort = bucket[order]
        counts = np.bincount(bsort, minlength=NBUK)
        gb = -(-counts // G)
        base = np.zeros(NBUK + 1, np.int64)
        np.cumsum(gb, out=base[1:])
        total_groups = int(base[-1])

    starts = np.zeros(NBUK, np.int64)
    starts[1:] = np.cumsum(counts)[:-1]
    rank = np.arange(order.size) - starts[bsort]
    gslot = (base[bsort] + rank // G) * G + rank % G   # global slot per point

    # per-group run offset (bf16 elements)
    off_bucket = (cell * ROW + t3 * (16 * 46)).astype(np.int32)
    idx_arr = np.zeros(NGRP, np.int32)
    # scatter per-bucket offsets to their group ranges
    occ = np.nonzero(gb)[0]
    obo = np.zeros(NBUK, np.int32)
    first = np.full(NBUK, -1, np.int64)
    first[bsort[::-1]] = order[::-1]          # first point of each bucket
    reps = np.repeat(first[occ], gb[occ])
    idx_arr[:total_groups] = off_bucket[reps]

    # device slot layout: group gid=(ch*128+p)*8+g ; slot row p*TOT + ch*48+g*6+s
    gid = gslot // G
    s_in = gslot % G
    chv = gid // 1024
    rem = gid % 1024
    pv = rem // 8
    gv = rem % 8
    col = chv * 48 + gv * G + s_in
    prow = pv * TOT + col

    pts4 = np.full((P * TOT, 4), 0.5, np.float32)
    pts4[:, 3] = 0.0
    pts4[prow, 0:3] = pts[order]
    pts4[prow, 3] = (TH * t3[order] - 1).astype(np.float32)

    slot_pt = np.full(P * TOT, -1, np.int64)
    slot_pt[prow] = order
    return idx_arr, pts4, slot_pt, (pv, col, order), overflow


def _eval_numpy(pts, g3):
    t = pts + np.float32(1.0)
    ti = np.trunc(t).astype(np.int64)
    x = t - ti.astype(np.float32)
    omx = np.float32(1.0) - x
    c0 = omx * omx * omx / np.float32(6)
    c1 = (np.float32(3) * (x - np.float32(2)) * x * x + np.float32(4)) / np.float32(6)
    c2 = (np.float32(-3) * omx * omx * (x + np.float32(1)) + np.float32(4)) / np.float32(6)
    c3 = x * x * x / np.float32(6)
    w = np.stack([c0, c1, c2, c3], axis=-1)
    out = np.zeros(pts.shape[0], np.float32)
    for i in range(4):
        for j in range(4):
            for k in range(4):
                out += (
                    g3[ti[:, 0] - 1 + i, ti[:, 1] - 1 + j, ti[:, 2] - 1 + k]
                    * w[:, 0, i] * w[:, 1, j] * w[:, 2, k]
                )
    return out


def host_inputs(pts, control_pts):
    pts = np.ascontiguousarray(pts, dtype=np.float32)
    g3 = np.ascontiguousarray(control_pts, np.float32).reshape(GRID, GRID, GRID)
    gt = np.ascontiguousarray(g3.transpose(1, 0, 2)).reshape(GRID * G2, 1)
    iota = np.tile(np.arange(46, dtype=np.float32), (P, 1))
    in_maps, metas = [], []
    for k in range(8):
        sl = pts[k * NPTS_CORE : (k + 1) * NPTS_CORE]
        idx_arr, pts4, slot_pt, devmap, overflow = _host_pack(sl)
        in_maps.append({
            "pts4": pts4,
            "idx": idx_arr.reshape(NGRP, 1),
            "gt": gt,
            "iota": iota,
        })
        metas.append((devmap, overflow, sl, g3))
    return in_maps, metas


def kernel(pts: np.ndarray, control_pts: np.ndarray) -> np.ndarray:
    from concourse.bass_utils import run_bass_kernel_spmd

    if "nc" not in _CACHE:
        _CACHE["nc"] = _build_program()
    nc = _CACHE["nc"]
    in_maps, metas = host_inputs(pts, control_pts)
    res = run_bass_kernel_spmd(nc, in_maps, core_ids=list(range(8)))
    outs = []
    for k in range(8):
        (pv, col, order), overflow, sl, g3 = metas[k]
        o = res.results[k]["out"]  # [128, TOT]
        r = np.zeros(NPTS_CORE, np.float32)
        r[order] = o[pv, col]
        if overflow.size:
            r[overflow] = _eval_numpy(sl[overflow], g3)
        outs.append(r)
    return np.concatenate(outs).reshape(-1, 1)
